# revision 15
# baseline (speedup 1.0000x reference)
"""Trainium2 Bass kernel: out = softmax(gelu_tanh(x @ W^T), axis=-1) + bias.

Full shapes: x [8192, 4096] f32, weight [4096, 4096] f32, bias [4096] f32.
Sharding: data-parallel over rows of x across 8 NeuronCores (1024 rows/core);
weight and bias replicated. Matmul runs in fp8e4m3 DoubleRow mode (2 k-subtiles
contracted per instruction) with fp32 PSUM accumulation; weight is pre-scaled
by 64 into [-1,1] (undone in the epilogue) to clear e4m3's min-normal boundary.

v2 structure (per core, MC=1024 rows = 8 m-tiles of 128):
  The whole fp8 weight (16MB) is resident in SBUF, streamed in exactly once.
  Phase 1 (rows 0-1) walks n-tile PAIRS outer so compute is paced to the
  weight DMA; phase 2 (rows 2-7) walks m-tiles outer with the full weight
  resident, so each row's softmax-normalize + bias-add + output DMA overlap
  the next row's matmuls. PSUM is used as [128,1024] 2-bank mega-tiles
  (ring of 4): the gelu+exp epilogue runs on 1024-wide APs (amortizing the
  ~352-cycle ACT fixed overhead) and each stationary x-tile LDWEIGHTS is
  shared by the 2 matmuls feeding the 2 banks. A pre-compile pass deletes
  the redundant second LDWEIGHTS of each pair (the PE keeps the loaded
  stationary), recovering ~50ns/matmul of PE front-end bandwidth.

gelu is computed with the exact tanh-approx constants of the reference via
Square/Tanh/Exp (all in the one `exp_and_others` ACT table -> single
ACT_TABLE_LOAD); softmax needs no max-subtraction because gelu output is
bounded in [-0.17, ~3.5] so exp cannot overflow.
"""

import sys

if "/opt/trn_rl_repo" not in sys.path:
    sys.path.insert(0, "/opt/trn_rl_repo")

import ml_dtypes
import numpy as np

import concourse.bass as bass
import concourse.tile as tile
from concourse import bacc, mybir
from concourse.bass_utils import run_bass_kernel_spmd

P = 128
GELU_A = 0.044715
GELU_C = 0.7978845608

FULL_M, FULL_K, FULL_N = 8192, 4096, 4096
NCORES = 8
MC = FULL_M // NCORES   # rows per core
MT = MC // P            # m-tiles per core (8)
KO = FULL_K // P        # k subtiles of 128 (32)
KP = KO // 2            # k pairs for DoubleRow (16)
NT = 512                # n tile (psum bank width in f32)
NTILES = FULL_N // NT   # 8
PH1 = 2                 # rows handled in the DMA-paced phase 1

W_SCALE = 64.0          # weight ~U(-1/64,1/64) sits at e4m3's min-normal
                        # boundary; scale into [-1,1], undo via ACT scale.
INV = 1.0 / W_SCALE

DEDUP_LDW = True        # delete redundant LDWEIGHTS (stationary reuse)


def _dedup_ldweights(nc):
    """Remove an InstLdweights whose weights AP equals the immediately
    preceding one's (the PE keeps the loaded stationary across matmuls).
    Deps of a removed load are merged into the next matmul."""
    remap = {}
    removed = 0
    for func in nc.m.functions:
        for block in func.blocks:
            new_insts = []
            last_sig = None
            pending = []
            for inst in block.instructions:
                if isinstance(inst, mybir.InstLdweights):
                    sig = (str(inst.ins[0]), str(inst.perf_mode))
                    if sig == last_sig:
                        pending.append(inst)
                        removed += 1
                        continue
                    last_sig = sig
                elif isinstance(inst, mybir.InstMatmult):
                    for d in pending:
                        inst.merge_dependencies_from(d)
                        remap[d.name] = inst.name
                    pending = []
                elif getattr(inst, "engine", None) == mybir.EngineType.PE:
                    if not isinstance(inst, mybir.InstEventSemaphore):
                        last_sig = None
                new_insts.append(inst)
            if pending:  # no matmul followed; keep them after all
                for d in pending:
                    new_insts.append(d)
                    removed -= 1
            block.instructions = new_insts
    if remap:
        for func in nc.m.functions:
            for block in func.blocks:
                for inst in block.instructions:
                    inst.remap_dependency_names(remap)
    return removed


def build_nc(dedup=DEDUP_LDW):
    f32 = mybir.dt.float32
    bf16 = mybir.dt.bfloat16
    fp8 = mybir.dt.float8e4

    nc = bacc.Bacc("TRN2", target_bir_lowering=False, debug=False)
    xt = nc.dram_tensor("xt", [MT, P, KO, P], fp8, kind="ExternalInput").ap()
    wt = nc.dram_tensor("wt", [NTILES, P, KO, NT], fp8, kind="ExternalInput").ap()
    bias = nc.dram_tensor("bias", [P, FULL_N], f32, kind="ExternalInput").ap()
    out = nc.dram_tensor("out", [P, MT, FULL_N], f32, kind="ExternalOutput").ap()

    with tile.TileContext(nc) as tc:
        with (
            tc.tile_pool(name="const", bufs=1) as const_pool,
            tc.tile_pool(name="x", bufs=3) as x_pool,
            tc.tile_pool(name="probs", bufs=2) as probs_pool,
            tc.tile_pool(name="tmp", bufs=1) as tmp_pool,
            tc.tile_pool(name="stat", bufs=1) as stat_pool,
            tc.tile_pool(name="stage", bufs=4) as stage_pool,
            tc.tile_pool(name="psum", bufs=3, space="PSUM") as psum_pool,
        ):
            w_sb = const_pool.tile([P, NTILES, KO, NT], fp8)
            bias_t = const_pool.tile([P, FULL_N], f32)
            # two chain-tmp sets so consecutive groups' gelu chains overlap
            # (a single set serializes the ~8us chain against the ~7us group
            # period and the backlog lands on the kernel tail)
            tAs = [tmp_pool.tile([P, 2 * NT], f32, name=f"tA{v}") for v in range(2)]
            tBs = [tmp_pool.tile([P, 2 * NT], f32, name=f"tB{v}") for v in range(2)]
            # 4 accum slots per row, +1 spare: the last row's final pair of
            # n-tiles runs as two 512-wide groups (shorter tail chain)
            sums = stat_pool.tile([P, MT * 4 + 1], f32)
            ssum = stat_pool.tile([P, MT], f32)
            recips = stat_pool.tile([P, MT], f32)

            x_tiles = {}

            def load_x(i, eng=None):
                x_tiles[i] = x_pool.tile([P, KO, P], fp8, tag="x", name=f"x{i}")
                (eng or nc.gpsimd).dma_start(x_tiles[i][:], xt[i])

            def load_w(j, h, nh, eng=None):
                hk = KO // nh
                (eng or nc.gpsimd).dma_start(
                    w_sb[:, j, h * hk : (h + 1) * hk, :],
                    wt[j, :, h * hk : (h + 1) * hk, :],
                )

            # phase-0 DMAs, in consumption order. The first-needed pieces (x0
            # halves + the first n-tile pair's k-quarters) go out on the ACT
            # engine's hw-DGE queue, which starts pumping earlier than the
            # gpsimd software queue; the bulk follows on gpsimd.
            x_tiles[0] = x_pool.tile([P, KO, P], fp8, tag="x", name="x0")
            nc.scalar.dma_start(x_tiles[0][:, : KO // 2, :], xt[0, :, : KO // 2, :])
            load_w(0, 0, 4, eng=nc.scalar)
            load_w(1, 0, 4, eng=nc.scalar)
            nc.scalar.dma_start(x_tiles[0][:, KO // 2 :, :], xt[0, :, KO // 2 :, :])
            load_w(0, 1, 4, eng=nc.scalar)
            load_w(1, 1, 4, eng=nc.scalar)
            load_x(1)
            for h in (2, 3):
                load_w(0, h, 4)
                load_w(1, h, 4)
            for jh in range(1, NTILES // 2):
                for h in range(2):
                    load_w(2 * jh, h, 2)
                    load_w(2 * jh + 1, h, 2)
                if jh == 2:
                    load_x(2)
            nc.gpsimd.dma_start(bias_t[:], bias[:])

            def mm_pair(ps, xi, jh):
                """16 k-pair steps; per step one stationary load feeds the
                two matmuls that fill the tile's two psum banks."""
                xv = x_tiles[xi]
                for k in range(KP):
                    for jj in range(2):
                        nc.tensor.matmul(
                            ps[:, jj * NT : (jj + 1) * NT],
                            xv[:, 2 * k : 2 * k + 2, :],
                            w_sb[:, 2 * jh + jj, 2 * k : 2 * k + 2, :],
                            start=(k == 0),
                            stop=(k == KP - 1),
                            perf_mode=mybir.MatmulPerfMode.DoubleRow,
                        )

            group_ctr = [0]

            def epilogue(ps, probs_t, i, jh):
                # p = exp(gelu(v)), gelu = 0.5*v*(1+tanh(C*(v+A*v^3)));
                # ps holds 64*v. Square/Tanh/Exp share one ACT table set.
                tA = tAs[group_ctr[0] % 2]
                tB = tBs[group_ctr[0] % 2]
                group_ctr[0] += 1
                nc.scalar.activation(
                    tA[:], ps[:], mybir.ActivationFunctionType.Square,
                    bias=0.0, scale=INV,
                )  # v^2
                nc.vector.tensor_scalar(
                    tB[:], tA[:], GELU_A * INV, INV,
                    mybir.AluOpType.mult, mybir.AluOpType.add,
                )  # (A*v^2+1)/64
                nc.vector.tensor_mul(tA[:], ps[:], tB[:])  # v + A*v^3
                nc.scalar.activation(
                    tB[:], tA[:], mybir.ActivationFunctionType.Tanh,
                    bias=0.0, scale=GELU_C,
                )
                nc.vector.scalar_tensor_tensor(
                    tA[:], tB[:], 1.0, ps[:],
                    mybir.AluOpType.add, mybir.AluOpType.mult,
                )  # (1+tanh)*64v
                sidx = i * 4 + jh
                nc.scalar.activation(
                    probs_t[:, jh * 2 * NT : (jh + 1) * 2 * NT], tA[:],
                    mybir.ActivationFunctionType.Exp,
                    bias=0.0, scale=0.5 * INV,
                    accum_out=sums[:, sidx : sidx + 1],
                )

            def normalize(i, probs_t, nsum=4, split=False):
                nc.vector.reduce_sum(
                    ssum[:, i : i + 1], sums[:, i * 4 : i * 4 + nsum],
                    axis=mybir.AxisListType.X,
                )
                nc.vector.reciprocal(recips[:, i : i + 1], ssum[:, i : i + 1])
                for q in range(4):
                    st = stage_pool.tile([P, 2 * NT], f32, tag="st", name="st")
                    nc.vector.scalar_tensor_tensor(
                        st[:],
                        probs_t[:, q * 2 * NT : (q + 1) * 2 * NT],
                        recips[:, i : i + 1],
                        bias_t[:, q * 2 * NT : (q + 1) * 2 * NT],
                        mybir.AluOpType.mult,
                        mybir.AluOpType.add,
                    )
                    # out-DMAs ride the ACT engine's hw-DGE queue, separate
                    # from the gpsimd queue carrying the x/w input stream
                    nc.scalar.dma_start(
                        out[:, i, q * 2 * NT : (q + 1) * 2 * NT], st[:]
                    )

            probs_tiles = {}

            def get_probs(i):
                probs_tiles[i] = probs_pool.tile(
                    [P, FULL_N], bf16, tag="probs", name=f"probs{i}"
                )
                return probs_tiles[i]

            # phase 1: rows 0..PH1-1, n-pair outer (paced to the w stream)
            for i in range(PH1):
                get_probs(i)
            for jh in range(NTILES // 2):
                for i in range(PH1):
                    ps = psum_pool.tile([P, 2 * NT], f32, tag="ps", name="ps")
                    mm_pair(ps, i, jh)
                    epilogue(ps, probs_tiles[i], i, jh)
            for i in range(PH1):
                normalize(i, probs_tiles[i])

            def epilogue512(ps, probs_t, j, sidx):
                tA = tAs[group_ctr[0] % 2]
                tB = tBs[group_ctr[0] % 2]
                group_ctr[0] += 1
                nc.scalar.activation(
                    tA[:, :NT], ps[:], mybir.ActivationFunctionType.Square,
                    bias=0.0, scale=INV,
                )
                nc.vector.tensor_scalar(
                    tB[:, :NT], tA[:, :NT], GELU_A * INV, INV,
                    mybir.AluOpType.mult, mybir.AluOpType.add,
                )
                nc.vector.tensor_mul(tA[:, :NT], ps[:], tB[:, :NT])
                nc.scalar.activation(
                    tB[:, :NT], tA[:, :NT], mybir.ActivationFunctionType.Tanh,
                    bias=0.0, scale=GELU_C,
                )
                nc.vector.scalar_tensor_tensor(
                    tA[:, :NT], tB[:, :NT], 1.0, ps[:],
                    mybir.AluOpType.add, mybir.AluOpType.mult,
                )
                nc.scalar.activation(
                    probs_t[:, j * NT : (j + 1) * NT], tA[:, :NT],
                    mybir.ActivationFunctionType.Exp,
                    bias=0.0, scale=0.5 * INV,
                    accum_out=sums[:, sidx : sidx + 1],
                )

            # phase 2: rows PH1..MT-1, m-tile outer over resident weight
            for i in range(PH1, MT):
                if i + 1 < MT:
                    load_x(i + 1)
                pt = get_probs(i)
                last = i == MT - 1
                for jh in range(3 if last else NTILES // 2):
                    ps = psum_pool.tile([P, 2 * NT], f32, tag="ps", name="ps")
                    mm_pair(ps, i, jh)
                    epilogue(ps, pt, i, jh)
                if last:
                    # final n-tile pair as two 512 groups: halves the exposed
                    # post-matmul chain latency on the kernel tail
                    pss = [
                        psum_pool.tile([P, NT], f32, tag="ps_s", name=f"ps_s{v}", bufs=2)
                        for v in range(2)
                    ]
                    xv = x_tiles[i]
                    for k in range(KP):
                        for jj in range(2):
                            nc.tensor.matmul(
                                pss[jj][:],
                                xv[:, 2 * k : 2 * k + 2, :],
                                w_sb[:, 6 + jj, 2 * k : 2 * k + 2, :],
                                start=(k == 0),
                                stop=(k == KP - 1),
                                perf_mode=mybir.MatmulPerfMode.DoubleRow,
                            )
                    for jj in range(2):
                        epilogue512(pss[jj], pt, 6 + jj, i * 4 + 3 + jj)
                    normalize(i, pt, nsum=5, split=True)
                else:
                    normalize(i, pt)

    if dedup:
        n = _dedup_ldweights(nc)
        assert n > 0, "ldweights dedup removed nothing"
    nc.compile()
    return nc


def pack_inputs(x, weight, bias):
    """Host-side shard + pack into the DMA-friendly layouts the kernel expects."""
    fp8_np = mybir.dt.np(mybir.dt.float8e4)
    w_src = weight * W_SCALE
    # wt[j, p, ko, n] = 64*weight[j*NT+n, ko*P+p]
    wt = np.ascontiguousarray(
        w_src.astype(fp8_np).reshape(NTILES, NT, KO, P).transpose(0, 3, 2, 1)
    )
    bias_b = np.ascontiguousarray(
        np.broadcast_to(bias.astype(np.float32)[None, :], (P, FULL_N))
    )
    in_maps = []
    for c in range(NCORES):
        xs = x[c * MC : (c + 1) * MC].astype(fp8_np)
        # xt[i, p, ko, m] = x_core[i*P+m, ko*P+p]
        xtc = np.ascontiguousarray(xs.reshape(MT, P, KO, P).transpose(0, 3, 2, 1))
        in_maps.append({"xt": xtc, "wt": wt, "bias": bias_b})
    return in_maps


def unpack_outputs(results):
    outs = []
    for res in results:
        o = np.asarray(res["out"])  # [P, MT, N]
        outs.append(o.transpose(1, 0, 2).reshape(MC, FULL_N))
    return np.concatenate(outs, axis=0)


_CACHE = {}


def _get_nc():
    if "nc" not in _CACHE:
        _CACHE["nc"] = build_nc()
    return _CACHE["nc"]


def _ensure_trace_env():
    """The agent image's antenv lacks axon_hooks, so NTFF tracing silently
    degrades. Register the ctypes-based hook ourselves, and neuter the S3
    artifact upload (no bucket access here)."""
    try:
        from antenv.axon_hooks import get_axon_ntff_profile_hook  # noqa: F401
    except ImportError:
        import types

        import antenv
        from trn_agent_boot.trn_boot import _ntff_profile_via_ctypes

        mod = types.ModuleType("antenv.axon_hooks")
        state = {"hook": _ntff_profile_via_ctypes("/opt/axon/libaxon_pjrt.so")}
        mod.set_axon_ntff_profile_hook = lambda h: state.__setitem__("hook", h)
        mod.get_axon_ntff_profile_hook = lambda: state["hook"]
        sys.modules["antenv.axon_hooks"] = mod
        antenv.axon_hooks = mod
    import concourse.bass_utils as bu

    bu.upload_artifacts = lambda tmpdir: f"local://{tmpdir}"


def kernel(x, weight, bias, trace=False, fp8=True):
    if trace:
        _ensure_trace_env()
    nc = _get_nc()
    in_maps = pack_inputs(
        np.asarray(x, dtype=np.float32),
        np.asarray(weight, dtype=np.float32),
        np.asarray(bias, dtype=np.float32),
    )
    res = run_bass_kernel_spmd(nc, in_maps, core_ids=list(range(NCORES)), trace=trace)
    out = unpack_outputs(res.results)
    if trace:
        return out, res
    return out


# revision 20
# speedup vs baseline: 1.0854x; 1.0854x over previous
"""Trainium2 Bass kernel: out = softmax(gelu_tanh(x @ W^T), axis=-1) + bias.

Full shapes: x [8192, 4096] f32, weight [4096, 4096] f32, bias [4096] f32.
Sharding: data-parallel over rows of x across 8 NeuronCores (1024 rows/core);
weight and bias replicated. Matmul runs in fp8e4m3 DoubleRow mode (2 k-subtiles
contracted per instruction) with fp32 PSUM accumulation; weight is pre-scaled
by 64 into [-1,1] (undone in the epilogue) to clear e4m3's min-normal boundary.

v2 structure (per core, MC=1024 rows = 8 m-tiles of 128):
  The whole fp8 weight (16MB) is resident in SBUF, streamed in exactly once.
  Phase 1 (rows 0-1) walks n-tile PAIRS outer so compute is paced to the
  weight DMA; phase 2 (rows 2-7) walks m-tiles outer with the full weight
  resident, so each row's softmax-normalize + bias-add + output DMA overlap
  the next row's matmuls. PSUM is used as [128,1024] 2-bank mega-tiles
  (ring of 4): the gelu+exp epilogue runs on 1024-wide APs (amortizing the
  ~352-cycle ACT fixed overhead) and each stationary x-tile LDWEIGHTS is
  shared by the 2 matmuls feeding the 2 banks. A pre-compile pass deletes
  the redundant second LDWEIGHTS of each pair (the PE keeps the loaded
  stationary), recovering ~50ns/matmul of PE front-end bandwidth.

gelu is computed with the exact tanh-approx constants of the reference via
Square/Tanh/Exp (all in the one `exp_and_others` ACT table -> single
ACT_TABLE_LOAD); softmax needs no max-subtraction because gelu output is
bounded in [-0.17, ~3.5] so exp cannot overflow.
"""

import sys

if "/opt/trn_rl_repo" not in sys.path:
    sys.path.insert(0, "/opt/trn_rl_repo")

import ml_dtypes
import numpy as np

import concourse.bass as bass
import concourse.tile as tile
from concourse import bacc, mybir
from concourse.bass_utils import run_bass_kernel_spmd

P = 128
GELU_A = 0.044715
GELU_C = 0.7978845608

FULL_M, FULL_K, FULL_N = 8192, 4096, 4096
NCORES = 8
MC = FULL_M // NCORES   # rows per core
MT = MC // P            # m-tiles per core (8)
KO = FULL_K // P        # k subtiles of 128 (32)
KP = KO // 2            # k pairs for DoubleRow (16)
NT = 512                # n tile (psum bank width in f32)
NTILES = FULL_N // NT   # 8
PH1 = 2                 # rows handled in the DMA-paced phase 1

W_SCALE = 64.0          # weight ~U(-1/64,1/64) sits at e4m3's min-normal
                        # boundary; scale into [-1,1], undo via ACT scale.
INV = 1.0 / W_SCALE

DEDUP_LDW = True        # delete redundant LDWEIGHTS (stationary reuse)


def _dedup_ldweights(nc):
    """Remove an InstLdweights whose weights AP equals the immediately
    preceding one's (the PE keeps the loaded stationary across matmuls).
    Deps of a removed load are merged into the next matmul."""
    remap = {}
    removed = 0
    for func in nc.m.functions:
        for block in func.blocks:
            new_insts = []
            last_sig = None
            pending = []
            for inst in block.instructions:
                if isinstance(inst, mybir.InstLdweights):
                    sig = (str(inst.ins[0]), str(inst.perf_mode))
                    if sig == last_sig:
                        pending.append(inst)
                        removed += 1
                        continue
                    last_sig = sig
                elif isinstance(inst, mybir.InstMatmult):
                    for d in pending:
                        inst.merge_dependencies_from(d)
                        remap[d.name] = inst.name
                    pending = []
                elif getattr(inst, "engine", None) == mybir.EngineType.PE:
                    if not isinstance(inst, mybir.InstEventSemaphore):
                        last_sig = None
                new_insts.append(inst)
            if pending:  # no matmul followed; keep them after all
                for d in pending:
                    new_insts.append(d)
                    removed -= 1
            block.instructions = new_insts
    if remap:
        for func in nc.m.functions:
            for block in func.blocks:
                for inst in block.instructions:
                    inst.remap_dependency_names(remap)
    return removed


def build_nc(dedup=DEDUP_LDW):
    f32 = mybir.dt.float32
    bf16 = mybir.dt.bfloat16
    fp8 = mybir.dt.float8e4

    nc = bacc.Bacc("TRN2", target_bir_lowering=False, debug=False)
    xt = nc.dram_tensor("xt", [MT, P, KO, P], fp8, kind="ExternalInput").ap()
    wt = nc.dram_tensor("wt", [NTILES, P, KO, NT], fp8, kind="ExternalInput").ap()
    bias = nc.dram_tensor("bias", [P, FULL_N], f32, kind="ExternalInput").ap()
    out = nc.dram_tensor("out", [P, MT, FULL_N], f32, kind="ExternalOutput").ap()

    with tile.TileContext(nc) as tc:
        with (
            tc.tile_pool(name="const", bufs=1) as const_pool,
            tc.tile_pool(name="x", bufs=3) as x_pool,
            tc.tile_pool(name="probs", bufs=2) as probs_pool,
            tc.tile_pool(name="tmp", bufs=1) as tmp_pool,
            tc.tile_pool(name="stat", bufs=1) as stat_pool,
            tc.tile_pool(name="stage", bufs=4) as stage_pool,
            tc.tile_pool(name="psum", bufs=4, space="PSUM") as psum_pool,
        ):
            w_sb = const_pool.tile([P, NTILES, KO, NT], fp8)
            bias_t = const_pool.tile([P, FULL_N], f32)
            # two chain-tmp sets so consecutive groups' gelu chains overlap
            # (a single set serializes the ~8us chain against the ~7us group
            # period and the backlog lands on the kernel tail)
            tAs = [tmp_pool.tile([P, 2 * NT], f32, name=f"tA{v}") for v in range(2)]
            tBs = [tmp_pool.tile([P, 2 * NT], f32, name=f"tB{v}") for v in range(2)]
            # 4 accum slots per row, +1 spare: the last row's final pair of
            # n-tiles runs as two 512-wide groups (shorter tail chain)
            sums = stat_pool.tile([P, MT * 4 + 1], f32)
            ssum = stat_pool.tile([P, MT], f32)
            recips = stat_pool.tile([P, MT], f32)

            x_tiles = {}

            def load_x(i, eng=None):
                x_tiles[i] = x_pool.tile([P, KO, P], fp8, tag="x", name=f"x{i}")
                (eng or nc.gpsimd).dma_start(x_tiles[i][:], xt[i])

            def load_w(j, h, nh, eng=None):
                hk = KO // nh
                (eng or nc.gpsimd).dma_start(
                    w_sb[:, j, h * hk : (h + 1) * hk, :],
                    wt[j, :, h * hk : (h + 1) * hk, :],
                )

            # phase-0 DMAs on the gpsimd queue, in consumption order: x0, the
            # first n-tile pair's weight in k-quarters (fast start), x1,
            # remaining pairs in k-halves, bias, x2. (The ACT hw-DGE queue
            # measured ~1.3us SLOWER to start than gpsimd's software queue,
            # so the head loads stay here.)
            load_x(0)
            for h in range(4):
                load_w(0, h, 4)
                load_w(1, h, 4)
                if h == 0:
                    load_x(1)
            for jh in range(1, NTILES // 2):
                for h in range(2):
                    load_w(2 * jh, h, 2)
                    load_w(2 * jh + 1, h, 2)
                if jh == 2:
                    load_x(2)
            nc.gpsimd.dma_start(bias_t[:], bias[:])

            def mm_pair(ps, xi, jh):
                """16 k-pair steps; per step one stationary load feeds the
                two matmuls that fill the tile's two psum banks."""
                xv = x_tiles[xi]
                for k in range(KP):
                    for jj in range(2):
                        nc.tensor.matmul(
                            ps[:, jj * NT : (jj + 1) * NT],
                            xv[:, 2 * k : 2 * k + 2, :],
                            w_sb[:, 2 * jh + jj, 2 * k : 2 * k + 2, :],
                            start=(k == 0),
                            stop=(k == KP - 1),
                            perf_mode=mybir.MatmulPerfMode.DoubleRow,
                        )

            group_ctr = [0]

            def epilogue(ps, probs_t, i, jh):
                # p = exp(gelu(v)), gelu = 0.5*v*(1+tanh(C*(v+A*v^3)));
                # ps holds 64*v. Square/Tanh/Exp share one ACT table set.
                tA = tAs[group_ctr[0] % 2]
                tB = tBs[group_ctr[0] % 2]
                group_ctr[0] += 1
                nc.scalar.activation(
                    tA[:], ps[:], mybir.ActivationFunctionType.Square,
                    bias=0.0, scale=INV,
                )  # v^2
                nc.vector.tensor_scalar(
                    tB[:], tA[:], GELU_A * INV, INV,
                    mybir.AluOpType.mult, mybir.AluOpType.add,
                )  # (A*v^2+1)/64
                nc.vector.tensor_mul(tA[:], ps[:], tB[:])  # v + A*v^3
                nc.scalar.activation(
                    tB[:], tA[:], mybir.ActivationFunctionType.Tanh,
                    bias=0.0, scale=GELU_C,
                )
                nc.vector.scalar_tensor_tensor(
                    tA[:], tB[:], 1.0, ps[:],
                    mybir.AluOpType.add, mybir.AluOpType.mult,
                )  # (1+tanh)*64v
                sidx = i * 4 + jh
                nc.scalar.activation(
                    probs_t[:, jh * 2 * NT : (jh + 1) * 2 * NT], tA[:],
                    mybir.ActivationFunctionType.Exp,
                    bias=0.0, scale=0.5 * INV,
                    accum_out=sums[:, sidx : sidx + 1],
                )

            def normalize(i, probs_t, nsum=4, split=False):
                nc.vector.reduce_sum(
                    ssum[:, i : i + 1], sums[:, i * 4 : i * 4 + nsum],
                    axis=mybir.AxisListType.X,
                )
                nc.vector.reciprocal(recips[:, i : i + 1], ssum[:, i : i + 1])
                if split:
                    # last row: finish with two 512 chunks so the final
                    # output DMA after the last normalize op is small
                    chunks = [(0, 2 * NT), (2 * NT, 2 * NT), (4 * NT, 2 * NT),
                              (6 * NT, NT), (7 * NT, NT)]
                else:
                    chunks = [(q * 2 * NT, 2 * NT) for q in range(4)]
                for off, width in chunks:
                    st = stage_pool.tile([P, 2 * NT], f32, tag="st", name="st")
                    nc.vector.scalar_tensor_tensor(
                        st[:, :width],
                        probs_t[:, off : off + width],
                        recips[:, i : i + 1],
                        bias_t[:, off : off + width],
                        mybir.AluOpType.mult,
                        mybir.AluOpType.add,
                    )
                    # out-DMAs ride the ACT engine's hw-DGE queue, separate
                    # from the gpsimd queue carrying the x/w input stream
                    nc.scalar.dma_start(
                        out[:, i, off : off + width], st[:, :width]
                    )

            probs_tiles = {}

            def get_probs(i):
                probs_tiles[i] = probs_pool.tile(
                    [P, FULL_N], bf16, tag="probs", name=f"probs{i}"
                )
                return probs_tiles[i]

            # phase 1: rows 0..PH1-1, n-pair outer (paced to the w stream)
            for i in range(PH1):
                get_probs(i)
            for jh in range(NTILES // 2):
                for i in range(PH1):
                    ps = psum_pool.tile([P, 2 * NT], f32, tag="ps", name="ps")
                    mm_pair(ps, i, jh)
                    epilogue(ps, probs_tiles[i], i, jh)
            for i in range(PH1):
                normalize(i, probs_tiles[i])

            def epilogue512(ps_ap, probs_t, j, sidx):
                tA = tAs[group_ctr[0] % 2]
                tB = tBs[group_ctr[0] % 2]
                group_ctr[0] += 1
                nc.scalar.activation(
                    tA[:, :NT], ps_ap, mybir.ActivationFunctionType.Square,
                    bias=0.0, scale=INV,
                )
                nc.vector.tensor_scalar(
                    tB[:, :NT], tA[:, :NT], GELU_A * INV, INV,
                    mybir.AluOpType.mult, mybir.AluOpType.add,
                )
                nc.vector.tensor_mul(tA[:, :NT], ps_ap, tB[:, :NT])
                nc.scalar.activation(
                    tB[:, :NT], tA[:, :NT], mybir.ActivationFunctionType.Tanh,
                    bias=0.0, scale=GELU_C,
                )
                nc.vector.scalar_tensor_tensor(
                    tA[:, :NT], tB[:, :NT], 1.0, ps_ap,
                    mybir.AluOpType.add, mybir.AluOpType.mult,
                )
                nc.scalar.activation(
                    probs_t[:, j * NT : (j + 1) * NT], tA[:, :NT],
                    mybir.ActivationFunctionType.Exp,
                    bias=0.0, scale=0.5 * INV,
                    accum_out=sums[:, sidx : sidx + 1],
                )

            # phase 2: rows PH1..MT-1, m-tile outer over resident weight
            for i in range(PH1, MT):
                if i + 1 < MT:
                    load_x(i + 1)
                pt = get_probs(i)
                last = i == MT - 1
                for jh in range(3 if last else NTILES // 2):
                    ps = psum_pool.tile([P, 2 * NT], f32, tag="ps", name="ps")
                    mm_pair(ps, i, jh)
                    epilogue(ps, pt, i, jh)
                if last:
                    # final n-tile pair as two sequential 512 groups (each in
                    # bank 0 of a main-tag tile, keeping the psum ring at 4):
                    # j6's chain overlaps j7's matmuls, so only j7's ~4.5us
                    # chain is exposed on the kernel tail instead of a full
                    # 1024-wide ~8us chain.
                    xv = x_tiles[i]
                    for jj in range(2):
                        ps = psum_pool.tile(
                            [P, 2 * NT], f32, tag="ps", name="ps"
                        )
                        for k in range(KP):
                            nc.tensor.matmul(
                                ps[:, :NT],
                                xv[:, 2 * k : 2 * k + 2, :],
                                w_sb[:, 6 + jj, 2 * k : 2 * k + 2, :],
                                start=(k == 0),
                                stop=(k == KP - 1),
                                perf_mode=mybir.MatmulPerfMode.DoubleRow,
                            )
                        epilogue512(ps[:, :NT], pt, 6 + jj, i * 4 + 3 + jj)
                    normalize(i, pt, nsum=5, split=True)
                else:
                    normalize(i, pt)

    if dedup:
        n = _dedup_ldweights(nc)
        assert n > 0, "ldweights dedup removed nothing"
    nc.compile()
    return nc


def pack_inputs(x, weight, bias):
    """Host-side shard + pack into the DMA-friendly layouts the kernel expects."""
    fp8_np = mybir.dt.np(mybir.dt.float8e4)
    w_src = weight * W_SCALE
    # wt[j, p, ko, n] = 64*weight[j*NT+n, ko*P+p]
    wt = np.ascontiguousarray(
        w_src.astype(fp8_np).reshape(NTILES, NT, KO, P).transpose(0, 3, 2, 1)
    )
    bias_b = np.ascontiguousarray(
        np.broadcast_to(bias.astype(np.float32)[None, :], (P, FULL_N))
    )
    in_maps = []
    for c in range(NCORES):
        xs = x[c * MC : (c + 1) * MC].astype(fp8_np)
        # xt[i, p, ko, m] = x_core[i*P+m, ko*P+p]
        xtc = np.ascontiguousarray(xs.reshape(MT, P, KO, P).transpose(0, 3, 2, 1))
        in_maps.append({"xt": xtc, "wt": wt, "bias": bias_b})
    return in_maps


def unpack_outputs(results):
    outs = []
    for res in results:
        o = np.asarray(res["out"])  # [P, MT, N]
        outs.append(o.transpose(1, 0, 2).reshape(MC, FULL_N))
    return np.concatenate(outs, axis=0)


_CACHE = {}


def _get_nc():
    if "nc" not in _CACHE:
        _CACHE["nc"] = build_nc()
    return _CACHE["nc"]


def _ensure_trace_env():
    """The agent image's antenv lacks axon_hooks, so NTFF tracing silently
    degrades. Register the ctypes-based hook ourselves, and neuter the S3
    artifact upload (no bucket access here)."""
    try:
        from antenv.axon_hooks import get_axon_ntff_profile_hook  # noqa: F401
    except ImportError:
        import types

        import antenv
        from trn_agent_boot.trn_boot import _ntff_profile_via_ctypes

        mod = types.ModuleType("antenv.axon_hooks")
        state = {"hook": _ntff_profile_via_ctypes("/opt/axon/libaxon_pjrt.so")}
        mod.set_axon_ntff_profile_hook = lambda h: state.__setitem__("hook", h)
        mod.get_axon_ntff_profile_hook = lambda: state["hook"]
        sys.modules["antenv.axon_hooks"] = mod
        antenv.axon_hooks = mod
    import concourse.bass_utils as bu

    bu.upload_artifacts = lambda tmpdir: f"local://{tmpdir}"


def kernel(x, weight, bias, trace=False, fp8=True):
    if trace:
        _ensure_trace_env()
    nc = _get_nc()
    in_maps = pack_inputs(
        np.asarray(x, dtype=np.float32),
        np.asarray(weight, dtype=np.float32),
        np.asarray(bias, dtype=np.float32),
    )
    res = run_bass_kernel_spmd(nc, in_maps, core_ids=list(range(NCORES)), trace=trace)
    out = unpack_outputs(res.results)
    if trace:
        return out, res
    return out


# revision 24
# speedup vs baseline: 1.0876x; 1.0020x over previous
"""Trainium2 Bass kernel: out = softmax(gelu_tanh(x @ W^T), axis=-1) + bias.

Full shapes: x [8192, 4096] f32, weight [4096, 4096] f32, bias [4096] f32.
Sharding: data-parallel over rows of x across 8 NeuronCores (1024 rows/core);
weight and bias replicated. Matmul runs in fp8e4m3 DoubleRow mode (2 k-subtiles
contracted per instruction) with fp32 PSUM accumulation; weight is pre-scaled
by 64 into [-1,1] (undone in the epilogue) to clear e4m3's min-normal boundary.

v2 structure (per core, MC=1024 rows = 8 m-tiles of 128):
  The whole fp8 weight (16MB) is resident in SBUF, streamed in exactly once.
  Phase 1 (rows 0-1) walks n-tile PAIRS outer so compute is paced to the
  weight DMA; phase 2 (rows 2-7) walks m-tiles outer with the full weight
  resident, so each row's softmax-normalize + bias-add + output DMA overlap
  the next row's matmuls. PSUM is used as [128,1024] 2-bank mega-tiles
  (ring of 4): the gelu+exp epilogue runs on 1024-wide APs (amortizing the
  ~352-cycle ACT fixed overhead) and each stationary x-tile LDWEIGHTS is
  shared by the 2 matmuls feeding the 2 banks. A pre-compile pass deletes
  the redundant second LDWEIGHTS of each pair (the PE keeps the loaded
  stationary), recovering ~50ns/matmul of PE front-end bandwidth.

gelu is computed with the exact tanh-approx constants of the reference via
Square/Tanh/Exp (all in the one `exp_and_others` ACT table -> single
ACT_TABLE_LOAD); softmax needs no max-subtraction because gelu output is
bounded in [-0.17, ~3.5] so exp cannot overflow.
"""

import sys

if "/opt/trn_rl_repo" not in sys.path:
    sys.path.insert(0, "/opt/trn_rl_repo")

import ml_dtypes
import numpy as np

import concourse.bass as bass
import concourse.tile as tile
from concourse import bacc, mybir
from concourse.bass_utils import run_bass_kernel_spmd

P = 128
GELU_A = 0.044715
GELU_C = 0.7978845608

FULL_M, FULL_K, FULL_N = 8192, 4096, 4096
NCORES = 8
MC = FULL_M // NCORES   # rows per core
MT = MC // P            # m-tiles per core (8)
KO = FULL_K // P        # k subtiles of 128 (32)
KP = KO // 2            # k pairs for DoubleRow (16)
NT = 512                # n tile (psum bank width in f32)
NTILES = FULL_N // NT   # 8
PH1 = 2                 # rows handled in the DMA-paced phase 1

W_SCALE = 64.0          # weight ~U(-1/64,1/64) sits at e4m3's min-normal
                        # boundary; scale into [-1,1], undo via ACT scale.
INV = 1.0 / W_SCALE

DEDUP_LDW = True        # delete redundant LDWEIGHTS (stationary reuse)


def _dedup_ldweights(nc):
    """Remove an InstLdweights whose weights AP equals the immediately
    preceding one's (the PE keeps the loaded stationary across matmuls).
    Deps of a removed load are merged into the next matmul."""
    remap = {}
    removed = 0
    for func in nc.m.functions:
        for block in func.blocks:
            new_insts = []
            last_sig = None
            pending = []
            for inst in block.instructions:
                if isinstance(inst, mybir.InstLdweights):
                    sig = (str(inst.ins[0]), str(inst.perf_mode))
                    if sig == last_sig:
                        pending.append(inst)
                        removed += 1
                        continue
                    last_sig = sig
                elif isinstance(inst, mybir.InstMatmult):
                    for d in pending:
                        inst.merge_dependencies_from(d)
                        remap[d.name] = inst.name
                    pending = []
                elif getattr(inst, "engine", None) == mybir.EngineType.PE:
                    if not isinstance(inst, mybir.InstEventSemaphore):
                        last_sig = None
                new_insts.append(inst)
            if pending:  # no matmul followed; keep them after all
                for d in pending:
                    new_insts.append(d)
                    removed -= 1
            block.instructions = new_insts
    if remap:
        for func in nc.m.functions:
            for block in func.blocks:
                for inst in block.instructions:
                    inst.remap_dependency_names(remap)
    return removed


def build_nc(dedup=DEDUP_LDW):
    f32 = mybir.dt.float32
    bf16 = mybir.dt.bfloat16
    fp8 = mybir.dt.float8e4

    nc = bacc.Bacc("TRN2", target_bir_lowering=False, debug=False)
    xt = nc.dram_tensor("xt", [MT, P, KO, P], fp8, kind="ExternalInput").ap()
    wt = nc.dram_tensor("wt", [NTILES, P, KO, NT], fp8, kind="ExternalInput").ap()
    bias = nc.dram_tensor("bias", [P, FULL_N], f32, kind="ExternalInput").ap()
    out = nc.dram_tensor("out", [P, MT, FULL_N], f32, kind="ExternalOutput").ap()

    with tile.TileContext(nc) as tc:
        with (
            tc.tile_pool(name="const", bufs=1) as const_pool,
            tc.tile_pool(name="x", bufs=3) as x_pool,
            tc.tile_pool(name="probs", bufs=2) as probs_pool,
            tc.tile_pool(name="tmp", bufs=1) as tmp_pool,
            tc.tile_pool(name="stat", bufs=1) as stat_pool,
            tc.tile_pool(name="stage", bufs=4) as stage_pool,
            tc.tile_pool(name="psum", bufs=4, space="PSUM") as psum_pool,
        ):
            w_sb = const_pool.tile([P, NTILES, KO, NT], fp8)
            bias_t = const_pool.tile([P, FULL_N], f32)
            # two chain-tmp sets so consecutive groups' gelu chains overlap
            # (a single set serializes the ~8us chain against the ~7us group
            # period and the backlog lands on the kernel tail)
            tAs = [tmp_pool.tile([P, 2 * NT], f32, name=f"tA{v}") for v in range(2)]
            tBs = [tmp_pool.tile([P, 2 * NT], f32, name=f"tB{v}") for v in range(2)]
            sums = stat_pool.tile([P, MT * 4], f32)
            ssum = stat_pool.tile([P, MT], f32)
            recips = stat_pool.tile([P, MT], f32)

            x_tiles = {}

            def load_x(i, eng=None):
                x_tiles[i] = x_pool.tile([P, KO, P], fp8, tag="x", name=f"x{i}")
                (eng or nc.gpsimd).dma_start(x_tiles[i][:], xt[i])

            def load_w(j, h, nh, eng=None):
                hk = KO // nh
                (eng or nc.gpsimd).dma_start(
                    w_sb[:, j, h * hk : (h + 1) * hk, :],
                    wt[j, :, h * hk : (h + 1) * hk, :],
                )

            # phase-0 DMAs on the gpsimd queue, in consumption order: x0 in
            # halves + the first n-tile pair's weight in k-eighths (the first
            # matmul needs only ~0.5MB), x1, remaining pairs in k-halves,
            # bias, x2. (The ACT hw-DGE queue measured ~1.3us SLOWER to start
            # than gpsimd's software queue, so the head loads stay here.)
            x_tiles[0] = x_pool.tile([P, KO, P], fp8, tag="x", name="x0")
            nc.gpsimd.dma_start(x_tiles[0][:, : KO // 2, :], xt[0, :, : KO // 2, :])
            load_w(0, 0, 8)
            load_w(1, 0, 8)
            nc.gpsimd.dma_start(x_tiles[0][:, KO // 2 :, :], xt[0, :, KO // 2 :, :])
            for h in range(1, 8):
                load_w(0, h, 8)
                load_w(1, h, 8)
                if h == 1:
                    load_x(1)
            for jh in range(1, NTILES // 2):
                for h in range(2):
                    load_w(2 * jh, h, 2)
                    load_w(2 * jh + 1, h, 2)
                if jh == 2:
                    load_x(2)
            nc.gpsimd.dma_start(bias_t[:], bias[:])

            def mm_pair(ps, xi, jh):
                """16 k-pair steps; per step one stationary load feeds the
                two matmuls that fill the tile's two psum banks."""
                xv = x_tiles[xi]
                for k in range(KP):
                    for jj in range(2):
                        nc.tensor.matmul(
                            ps[:, jj * NT : (jj + 1) * NT],
                            xv[:, 2 * k : 2 * k + 2, :],
                            w_sb[:, 2 * jh + jj, 2 * k : 2 * k + 2, :],
                            start=(k == 0),
                            stop=(k == KP - 1),
                            perf_mode=mybir.MatmulPerfMode.DoubleRow,
                        )

            group_ctr = [0]

            def epilogue(ps, probs_t, i, jh):
                # p = exp(gelu(v)), gelu = 0.5*v*(1+tanh(C*(v+A*v^3)));
                # ps holds 64*v. Square/Tanh/Exp share one ACT table set.
                tA = tAs[group_ctr[0] % 2]
                tB = tBs[group_ctr[0] % 2]
                group_ctr[0] += 1
                nc.scalar.activation(
                    tA[:], ps[:], mybir.ActivationFunctionType.Square,
                    bias=0.0, scale=INV,
                )  # v^2
                nc.vector.tensor_scalar(
                    tB[:], tA[:], GELU_A * INV, INV,
                    mybir.AluOpType.mult, mybir.AluOpType.add,
                )  # (A*v^2+1)/64
                nc.vector.tensor_mul(tA[:], ps[:], tB[:])  # v + A*v^3
                nc.scalar.activation(
                    tB[:], tA[:], mybir.ActivationFunctionType.Tanh,
                    bias=0.0, scale=GELU_C,
                )
                nc.vector.scalar_tensor_tensor(
                    tA[:], tB[:], 1.0, ps[:],
                    mybir.AluOpType.add, mybir.AluOpType.mult,
                )  # (1+tanh)*64v
                sidx = i * 4 + jh
                nc.scalar.activation(
                    probs_t[:, jh * 2 * NT : (jh + 1) * 2 * NT], tA[:],
                    mybir.ActivationFunctionType.Exp,
                    bias=0.0, scale=0.5 * INV,
                    accum_out=sums[:, sidx : sidx + 1],
                )

            def normalize(i, probs_t, nsum=4, split=False):
                nc.vector.reduce_sum(
                    ssum[:, i : i + 1], sums[:, i * 4 : i * 4 + nsum],
                    axis=mybir.AxisListType.X,
                )
                nc.vector.reciprocal(recips[:, i : i + 1], ssum[:, i : i + 1])
                if split:
                    # last row: finish with two 512 chunks so the final
                    # output DMA after the last normalize op is small
                    chunks = [(0, 2 * NT), (2 * NT, 2 * NT), (4 * NT, 2 * NT),
                              (6 * NT, NT), (7 * NT, NT)]
                else:
                    chunks = [(q * 2 * NT, 2 * NT) for q in range(4)]
                for off, width in chunks:
                    st = stage_pool.tile([P, 2 * NT], f32, tag="st", name="st")
                    nc.vector.scalar_tensor_tensor(
                        st[:, :width],
                        probs_t[:, off : off + width],
                        recips[:, i : i + 1],
                        bias_t[:, off : off + width],
                        mybir.AluOpType.mult,
                        mybir.AluOpType.add,
                    )
                    # out-DMAs ride the ACT engine's hw-DGE queue, separate
                    # from the gpsimd queue carrying the x/w input stream
                    nc.scalar.dma_start(
                        out[:, i, off : off + width], st[:, :width]
                    )

            probs_tiles = {}

            def get_probs(i):
                probs_tiles[i] = probs_pool.tile(
                    [P, FULL_N], bf16, tag="probs", name=f"probs{i}"
                )
                return probs_tiles[i]

            # phase 1: rows 0..PH1-1, n-pair outer (paced to the w stream)
            for i in range(PH1):
                get_probs(i)
            for jh in range(NTILES // 2):
                for i in range(PH1):
                    ps = psum_pool.tile([P, 2 * NT], f32, tag="ps", name="ps")
                    mm_pair(ps, i, jh)
                    epilogue(ps, probs_tiles[i], i, jh)
            for i in range(PH1):
                normalize(i, probs_tiles[i])

            # phase 2: rows PH1..MT-1, m-tile outer over resident weight
            for i in range(PH1, MT):
                if i + 1 < MT:
                    load_x(i + 1)
                pt = get_probs(i)
                for jh in range(NTILES // 2):
                    ps = psum_pool.tile([P, 2 * NT], f32, tag="ps", name="ps")
                    mm_pair(ps, i, jh)
                    epilogue(ps, pt, i, jh)
                normalize(i, pt, split=(i == MT - 1))

    if dedup:
        n = _dedup_ldweights(nc)
        assert n > 0, "ldweights dedup removed nothing"
    nc.compile()
    return nc


def pack_inputs(x, weight, bias):
    """Host-side shard + pack into the DMA-friendly layouts the kernel expects."""
    fp8_np = mybir.dt.np(mybir.dt.float8e4)
    w_src = weight * W_SCALE
    # wt[j, p, ko, n] = 64*weight[j*NT+n, ko*P+p]
    wt = np.ascontiguousarray(
        w_src.astype(fp8_np).reshape(NTILES, NT, KO, P).transpose(0, 3, 2, 1)
    )
    bias_b = np.ascontiguousarray(
        np.broadcast_to(bias.astype(np.float32)[None, :], (P, FULL_N))
    )
    in_maps = []
    for c in range(NCORES):
        xs = x[c * MC : (c + 1) * MC].astype(fp8_np)
        # xt[i, p, ko, m] = x_core[i*P+m, ko*P+p]
        xtc = np.ascontiguousarray(xs.reshape(MT, P, KO, P).transpose(0, 3, 2, 1))
        in_maps.append({"xt": xtc, "wt": wt, "bias": bias_b})
    return in_maps


def unpack_outputs(results):
    outs = []
    for res in results:
        o = np.asarray(res["out"])  # [P, MT, N]
        outs.append(o.transpose(1, 0, 2).reshape(MC, FULL_N))
    return np.concatenate(outs, axis=0)


_CACHE = {}


def _get_nc():
    if "nc" not in _CACHE:
        _CACHE["nc"] = build_nc()
    return _CACHE["nc"]


def _ensure_trace_env():
    """The agent image's antenv lacks axon_hooks, so NTFF tracing silently
    degrades. Register the ctypes-based hook ourselves, and neuter the S3
    artifact upload (no bucket access here)."""
    try:
        from antenv.axon_hooks import get_axon_ntff_profile_hook  # noqa: F401
    except ImportError:
        import types

        import antenv
        from trn_agent_boot.trn_boot import _ntff_profile_via_ctypes

        mod = types.ModuleType("antenv.axon_hooks")
        state = {"hook": _ntff_profile_via_ctypes("/opt/axon/libaxon_pjrt.so")}
        mod.set_axon_ntff_profile_hook = lambda h: state.__setitem__("hook", h)
        mod.get_axon_ntff_profile_hook = lambda: state["hook"]
        sys.modules["antenv.axon_hooks"] = mod
        antenv.axon_hooks = mod
    import concourse.bass_utils as bu

    bu.upload_artifacts = lambda tmpdir: f"local://{tmpdir}"


def kernel(x, weight, bias, trace=False, fp8=True):
    if trace:
        _ensure_trace_env()
    nc = _get_nc()
    in_maps = pack_inputs(
        np.asarray(x, dtype=np.float32),
        np.asarray(weight, dtype=np.float32),
        np.asarray(bias, dtype=np.float32),
    )
    res = run_bass_kernel_spmd(nc, in_maps, core_ids=list(range(NCORES)), trace=trace)
    out = unpack_outputs(res.results)
    if trace:
        return out, res
    return out
